# revision 14
# baseline (speedup 1.0000x reference)
"""Trainium2 Bass kernel for nn_CayleyFilter (gnn_message_passing).

Math: the reference's Jacobi step degenerates — its SpMM terms cancel
algebraically:
    tr = (offr + diag*zr) + zi - offr == diag*zr + zi   (+- fp rounding noise)
    ti = (offi + diag*zi) - zr - offi == diag*zi - zr
so each Cayley iteration is an elementwise multiply by the per-node
unit-modulus complex scalar s_p = (d_p - i)^2 / (d_p^2 + 1).  Hence
    z_k = s^k (x)   and the whole module collapses to one GEMM:
    out[(n,p), o] = sum_{g,c} coef_g[p] * x[n,c,p] * W2[(g,c), o]
with coef_g in {Re(s^k), Im(s^k)} and W2 = 2*[Wr; Wi].

v2 layout (vs the fp32 batch-parallel v1 at 62.6us/iter):
  - shard over p (M) across the 8 cores: each core owns a 512-column
    slice of the graph for ALL 32 batches.  coef shrinks 8x per core.
  - fp16 operands end-to-end: DVE tensor_tensor runs in 2x_1P mode
    (the fp32 build ran 1x and dominated), DMA volume halves, the
    fp16 matmul is full-rate, PSUM accumulation stays fp32.
  - contraction = exactly the 16 nontrivial groups (k=1..8, re/im) =
    4 chunks of 128 rows; the k=0-real group (coef==1) is a direct
    K=32 matmul on x (no elementwise work, no copy); k=0-imag (==0)
    is dropped entirely.
  - matmuls col-tiled in pairs via tile_position (0,0)/(0,64): two
    batch-items share the 128 PSUM partitions -> ~2x TensorE.
  - fast path when diag_L is constant (e.g. all-ones): coefs are
    p-independent and fold into W on the host; the module collapses
    to a single K=32 GEMM, row+col-tiled 8 ways.
"""

import os

import numpy as np

N, C, M, MSIDE, COUT, ORDER = 32, 32, 4096, 64, 64, 8
NCORES = 8
KTOT = ORDER + 1              # 9
NGRP = 2 * ORDER              # 16 nontrivial coefficient groups
NCHUNK = 4                    # contraction chunks of 128 rows
PT = 512                      # p-columns per core (M / NCORES)
NPAIR = N // 2                # 16 col-tiled item pairs

_STATE = {}
LAST_RESULTS = None


def _make_nc(loop_reps=0, dve_chunks=(3, 3, 3, 4)):
    """General-path SPMD program (one 512-wide p-slice per core).

    loop_reps>0 wraps the compute in a hardware For_i loop (bench-only).
    dve_chunks[i % len]: chunks built on VectorE for item i; the rest
    of the 4 chunks go to GpSimdE.
    """
    import contextlib

    import concourse.bass as bass
    import concourse.mybir as mybir
    from concourse.tile import TileContext

    f32 = mybir.dt.float32
    f16 = mybir.dt.float16

    nc = bass.Bass()
    x_d = nc.dram_tensor("x", [128, N, PT], f16, kind="ExternalInput")
    coef_d = nc.dram_tensor("coef", [128, NCHUNK, PT], f16, kind="ExternalInput")
    w_d = nc.dram_tensor("w", [128, NCHUNK, COUT], f16, kind="ExternalInput")
    wt_d = nc.dram_tensor("wt", [64, COUT], f16, kind="ExternalInput")
    out_d = nc.dram_tensor("out", [NPAIR, 128, PT], f16, kind="ExternalOutput")

    with TileContext(nc) as tc:
        with (
            tc.tile_pool(name="const", bufs=1) as cpool,
            tc.tile_pool(name="ft", bufs=4) as ftpool,
            tc.tile_pool(name="osb", bufs=4) as opool,
            tc.tile_pool(name="ps", bufs=4, space="PSUM") as pspool,
        ):
            # weights + coef first: small, one-time setup constants
            w_sb = cpool.tile([128, NCHUNK * COUT], f16, tag="w")
            wt_sb = cpool.tile([64, COUT], f16, tag="wt")
            coef_sb = cpool.tile([128, NCHUNK * PT], f16, tag="coef")

            nc.sync.dma_start(
                w_sb[:, :].rearrange("p (q o) -> p q o", q=NCHUNK), w_d[:]
            )
            nc.sync.dma_start(wt_sb[:, :], wt_d[:])
            nc.sync.dma_start(
                coef_sb[:, :].rearrange("p (q m) -> p q m", q=NCHUNK), coef_d[:]
            )
            # x streamed in 4-item blocks (per-block tiles so the bench
            # loop's reloads pipeline at block granularity)
            XB = 4
            xb = [cpool.tile([128, XB * PT], f16, tag=f"xb{b}", name=f"xb{b}")
                  for b in range(N // XB)]

            coef3d = coef_sb[:, :].rearrange("r (q m) -> r q m", q=NCHUNK)

            loop_cm = tc.For_i(0, loop_reps, 1) if loop_reps else contextlib.nullcontext()
            with loop_cm:
                for b in range(N // XB):
                    nc.sync.dma_start(
                        xb[b][:, :].rearrange("p (n m) -> p n m", n=XB),
                        x_d[:, b * XB:(b + 1) * XB, :],
                    )
                for pr in range(NPAIR):
                    items = (2 * pr, 2 * pr + 1)
                    fts = []
                    for j, it in enumerate(items):
                        ndve = dve_chunks[it % len(dve_chunks)]
                        ngp = NCHUNK - ndve
                        xs = xb[it // XB][:, (it % XB) * PT:(it % XB + 1) * PT]

                        ft = ftpool.tile([128, NCHUNK * PT], f16, name=f"ft{j}")
                        nc.vector.tensor_mul(
                            ft[:, : ndve * PT].rearrange(
                                "r (q p) -> r q p", q=ndve
                            ),
                            xs.unsqueeze(1).broadcast_to([128, ndve, PT]),
                            coef3d[:, :ndve, :],
                        )
                        if ngp:
                            nc.gpsimd.tensor_mul(
                                ft[:, ndve * PT:].rearrange(
                                    "r (q p) -> r q p", q=ngp
                                ),
                                xs.unsqueeze(1).broadcast_to([128, ngp, PT]),
                                coef3d[:, ndve:, :],
                            )
                        fts.append(ft)

                    ps = pspool.tile([128, PT], f32)
                    halves = (ps[0:64, :], ps[64:128, :])
                    for q in range(NCHUNK):
                        for j in range(2):
                            nc.tensor.matmul(
                                halves[j],
                                w_sb[:, q * COUT:(q + 1) * COUT],
                                fts[j][:, q * PT:(q + 1) * PT],
                                start=(q == 0),
                                stop=False,
                                tile_position=(0, 64 * j),
                                skip_group_check=True,
                            )
                    # k=0-real tail: K=32 matmul straight on x, row-tiled
                    # so the two halves' tails run concurrently
                    for j, it in enumerate(items):
                        nc.tensor.matmul(
                            halves[j],
                            wt_sb[32 * j:32 * (j + 1), :],
                            xb[it // XB][
                                32 * j:32 * (j + 1),
                                (it % XB) * PT:(it % XB + 1) * PT,
                            ],
                            start=False,
                            stop=True,
                            tile_position=(32 * j, 64 * j),
                            skip_group_check=True,
                        )
                    osb = opool.tile([128, PT], f16)
                    nc.scalar.copy(osb[:, :], ps[:, :])
                    nc.sync.dma_start(out_d[pr], osb[:, :])

    import bass_rust
    bass_rust.generate_event_semaphores(nc)
    return nc


def _make_nc_const(loop_reps=0):
    """Fast path: diag_L constant => coefs fold into W; one K=32 GEMM.

    x packed [128 = 4 n-sub x 32 c, N/4 n-grp, PT]: the 4 n-subs are 4
    row-groups of the PE array, pairs of items are 2 col-groups -> 8
    concurrent K=32 matmuls.
    """
    import contextlib

    import concourse.bass as bass
    import concourse.mybir as mybir
    from concourse.tile import TileContext

    f32 = mybir.dt.float32
    f16 = mybir.dt.float16
    NG = N // 4               # 8 n-groups of 4 items

    XBLK = (1, 3, 4)          # x-load taper: tiny first block -> early MMs
    OBLK = (2, 3, 3)          # out-batch taper (n-groups per DMA): short tail

    nc = bass.Bass()
    x_d = nc.dram_tensor("x", [128, NG, PT], f16, kind="ExternalInput")
    w_d = nc.dram_tensor("w", [128, COUT], f16, kind="ExternalInput")
    out_d = nc.dram_tensor("out", [128, NG * 2 * PT], f16, kind="ExternalOutput")

    with TileContext(nc) as tc:
        with (
            tc.tile_pool(name="const", bufs=1) as cpool,
            tc.tile_pool(name="osb", bufs=3) as opool,
            tc.tile_pool(name="ps", bufs=4, space="PSUM") as pspool,
        ):
            # w: Wfold replicated on all 4 row-groups (partitions 4x32)
            w_sb = cpool.tile([128, COUT], f16, tag="w")
            xs = [cpool.tile([128, nb * PT], f16, tag=f"x{t}", name=f"x{t}")
                  for t, nb in enumerate(XBLK)]
            xoff = [sum(XBLK[:t]) for t in range(len(XBLK))]

            def xtile(b):     # -> (tile, col-offset) holding n-group b
                for t in range(len(XBLK) - 1, -1, -1):
                    if b >= xoff[t]:
                        return xs[t], (b - xoff[t]) * PT
                raise AssertionError

            nc.sync.dma_start(w_sb[:, :], w_d[:])

            loop_cm = tc.For_i(0, loop_reps, 1) if loop_reps else contextlib.nullcontext()
            with loop_cm:
                for t, nb in enumerate(XBLK):
                    nc.sync.dma_start(
                        xs[t][:, :].rearrange("p (u m) -> p u m", u=nb),
                        x_d[:, xoff[t]:xoff[t] + nb, :],
                    )
                bdone = 0
                for obatch, nob in enumerate(OBLK):   # nob n-groups per out-DMA
                    osb = opool.tile([128, nob * 2 * PT], f16)
                    for u in range(nob):
                        b = bdone + u                 # items 4b..4b+3
                        xt, xo = xtile(b)
                        pss = [pspool.tile([128, PT], f32, name=f"ps{k}")
                               for k in range(2)]
                        for sub in range(4):          # item 4b+sub; pair k=sub//2
                            ps = pss[sub // 2]
                            nc.tensor.matmul(
                                ps[64 * (sub % 2):64 * (sub % 2 + 1), :],
                                w_sb[32 * sub:32 * (sub + 1), :],
                                xt[32 * sub:32 * (sub + 1), xo:xo + PT],
                                start=True,
                                stop=True,
                                tile_position=(32 * sub, 64 * (sub % 2)),
                                skip_group_check=True,
                            )
                        # evac split: ScalarE + (idle) VectorE in parallel
                        o0 = osb[:, (2 * u) * PT:(2 * u + 1) * PT]
                        o1 = osb[:, (2 * u + 1) * PT:(2 * u + 2) * PT]
                        nc.scalar.copy(o0, pss[0][:, :])
                        nc.vector.tensor_copy(o1, pss[1][:, :])
                    nc.sync.dma_start(
                        out_d[:, bdone * 2 * PT:(bdone + nob) * 2 * PT],
                        osb[:, :],
                    )
                    bdone += nob

    import bass_rust
    bass_rust.generate_event_semaphores(nc)
    return nc


def _coefs(diag_L):
    """[16, M] nontrivial coefficient rows: Re(s^k), Im(s^k), k=1..8."""
    d = np.asarray(diag_L, dtype=np.float64)
    s = (d - 1j) ** 2 / (d * d + 1.0)
    out = np.empty((NGRP, d.shape[0]), dtype=np.float64)
    ck = s.copy()
    for k in range(ORDER):
        out[k] = ck.real
        out[ORDER + k] = ck.imag
        ck = ck * s
    return out


def _prep_host(x, real_weights, imag_weights, diag_L):
    x3 = np.asarray(x, dtype=np.float32).reshape(N, C, M)
    wr = np.asarray(real_weights, dtype=np.float64).reshape(KTOT, C, COUT)
    wi = np.asarray(imag_weights, dtype=np.float64).reshape(KTOT, C, COUT)
    coefs = _coefs(diag_L)

    # xr[g*32+c, n, p] = x[n, c, p]  (x4 over partition groups)
    xt = x3.transpose(1, 0, 2).astype(np.float16)           # [c, n, p]
    xr = np.broadcast_to(xt[None], (4, C, N, M)).reshape(128, N, M)

    # coef[gs*32+c, q, p] = coefs[4q+gs, p]
    cf = np.repeat(coefs.reshape(NCHUNK, 4, 1, M), C, axis=2)   # [q, gs, c, p]
    cf = np.ascontiguousarray(
        cf.transpose(1, 2, 0, 3).reshape(128, NCHUNK, M)
    ).astype(np.float16)

    # w[gs*32+c, q, o] = 2 * W_{4q+gs}[c, o]; groups = [re k=1..8, im k=1..8]
    wall = 2.0 * np.concatenate([wr[1:], wi[1:]], axis=0)       # [16, c, o]
    w = np.ascontiguousarray(
        wall.reshape(NCHUNK, 4, C, COUT).transpose(1, 2, 0, 3).reshape(
            128, NCHUNK, COUT
        )
    ).astype(np.float16)

    # tail: k=0-real (coef==1) on two row-group copies
    wt = np.ascontiguousarray(
        np.broadcast_to(2.0 * wr[0], (2, C, COUT)).reshape(64, COUT)
    ).astype(np.float16)
    return xr, cf, w, wt


def _prep_host_const(x, real_weights, imag_weights, diag_L):
    x3 = np.asarray(x, dtype=np.float32).reshape(N, C, M)
    wr = np.asarray(real_weights, dtype=np.float64).reshape(KTOT, C, COUT)
    wi = np.asarray(imag_weights, dtype=np.float64).reshape(KTOT, C, COUT)
    coefs = _coefs(diag_L[:1])                                  # [16, 1]
    # Wfold[c, o] = 2*Wr0 + sum_g coef_g * W_g
    wfold = 2.0 * wr[0] + np.tensordot(
        coefs[:, 0], 2.0 * np.concatenate([wr[1:], wi[1:]], axis=0), axes=(0, 0)
    )
    # x packed [sub*32+c, ngrp, p], n = ngrp*4 + sub
    xt = x3.reshape(N // 4, 4, C, M).transpose(1, 2, 0, 3)      # [sub, c, ng, p]
    xp = np.ascontiguousarray(xt.reshape(128, N // 4, M)).astype(np.float16)
    wf = np.ascontiguousarray(
        np.broadcast_to(wfold, (4, C, COUT)).reshape(128, COUT)
    ).astype(np.float16)
    return xp, wf


def _in_map_general(args, i):
    xr, cf, w, wt = args
    sl = slice(PT * i, PT * (i + 1))
    return {
        "x": np.ascontiguousarray(xr[:, :, sl]),
        "coef": np.ascontiguousarray(cf[:, :, sl]),
        "w": w,
        "wt": wt,
    }


def _in_map_const(args, i):
    xp, wf = args
    sl = slice(PT * i, PT * (i + 1))
    return {"x": np.ascontiguousarray(xp[:, :, sl]), "w": wf}


def _assemble(results, const):
    out = np.empty((N, M, COUT), dtype=np.float32)
    for i in range(NCORES):
        o = np.asarray(results[i]["out"], dtype=np.float32)
        if const:
            # out[(half, o), (b, k, p)]: n = 4b + 2k + half
            o = o.reshape(2, COUT, 8, 2, PT).transpose(2, 3, 0, 4, 1)
        else:
            # out[pair, (half, o), p]: n = 2*pair + half
            o = o.reshape(NPAIR, 2, COUT, PT).transpose(0, 1, 3, 2)
        out[:, PT * i:PT * (i + 1), :] = o.reshape(N, PT, COUT)
    return out.reshape(N, MSIDE, MSIDE, COUT)


def kernel(x, real_weights, imag_weights, diag_L, vals, rows, cols):
    global LAST_RESULTS
    from concourse.bass_utils import run_bass_kernel_spmd

    diag = np.asarray(diag_L, dtype=np.float32)
    const_diag = bool(np.all(diag == diag[0]))
    if os.environ.get("CAYLEY_FORCE_GENERAL"):
        const_diag = False

    if const_diag:
        args = _prep_host_const(x, real_weights, imag_weights, diag)
        if "nc_const" not in _STATE:
            _STATE["nc_const"] = _make_nc_const()
        nc = _STATE["nc_const"]
        in_maps = [_in_map_const(args, i) for i in range(NCORES)]
    else:
        args = _prep_host(x, real_weights, imag_weights, diag)
        if "nc" not in _STATE:
            _STATE["nc"] = _make_nc()
        nc = _STATE["nc"]
        in_maps = [_in_map_general(args, i) for i in range(NCORES)]

    res = run_bass_kernel_spmd(nc, in_maps, list(range(NCORES)))
    LAST_RESULTS = res
    return _assemble(res.results, const_diag)


# revision 18
# speedup vs baseline: 1.0766x; 1.0766x over previous
"""Trainium2 Bass kernel for nn_CayleyFilter (gnn_message_passing).

Math: the reference's Jacobi step degenerates — its SpMM terms cancel
algebraically:
    tr = (offr + diag*zr) + zi - offr == diag*zr + zi   (+- fp rounding noise)
    ti = (offi + diag*zi) - zr - offi == diag*zi - zr
so each Cayley iteration is an elementwise multiply by the per-node
unit-modulus complex scalar s_p = (d_p - i)^2 / (d_p^2 + 1).  Hence
    z_k = s^k (x)   and the whole module collapses to one GEMM:
    out[(n,p), o] = sum_{g,c} coef_g[p] * x[n,c,p] * W2[(g,c), o]
with coef_g in {Re(s^k), Im(s^k)} and W2 = 2*[Wr; Wi].

v2 layout (vs the fp32 batch-parallel v1 at 62.6us/iter):
  - shard over p (M) across the 8 cores: each core owns a 512-column
    slice of the graph for ALL 32 batches.  coef shrinks 8x per core.
  - fp16 operands end-to-end: DVE tensor_tensor runs in 2x_1P mode
    (the fp32 build ran 1x and dominated), DMA volume halves, the
    fp16 matmul is full-rate, PSUM accumulation stays fp32.
  - contraction = exactly the 16 nontrivial groups (k=1..8, re/im) =
    4 chunks of 128 rows; the k=0-real group (coef==1) is a direct
    K=32 matmul on x (no elementwise work, no copy); k=0-imag (==0)
    is dropped entirely.
  - matmuls col-tiled in pairs via tile_position (0,0)/(0,64): two
    batch-items share the 128 PSUM partitions -> ~2x TensorE.
  - fast path when diag_L is constant (e.g. all-ones): coefs are
    p-independent and fold into W on the host; the module collapses
    to a single K=32 GEMM, row+col-tiled 8 ways.
"""

import os

import numpy as np

N, C, M, MSIDE, COUT, ORDER = 32, 32, 4096, 64, 64, 8
NCORES = 8
KTOT = ORDER + 1              # 9
NGRP = 2 * ORDER              # 16 nontrivial coefficient groups
NCHUNK = 4                    # contraction chunks of 128 rows
PT = 512                      # p-columns per core (M / NCORES)
NPAIR = N // 2                # 16 col-tiled item pairs

_STATE = {}
LAST_RESULTS = None


def _make_nc(loop_reps=0, dve_chunks=(3, 3, 3, 4)):
    """General-path SPMD program (one 512-wide p-slice per core).

    loop_reps>0 wraps the compute in a hardware For_i loop (bench-only).
    dve_chunks[i % len]: chunks built on VectorE for item i; the rest
    of the 4 chunks go to GpSimdE.
    """
    import contextlib

    import concourse.bass as bass
    import concourse.mybir as mybir
    from concourse.tile import TileContext

    f32 = mybir.dt.float32
    f16 = mybir.dt.float16

    nc = bass.Bass()
    x_d = nc.dram_tensor("x", [128, N, PT], f16, kind="ExternalInput")
    coef_d = nc.dram_tensor("coef", [128, NCHUNK, PT], f16, kind="ExternalInput")
    w_d = nc.dram_tensor("w", [128, NCHUNK, COUT], f16, kind="ExternalInput")
    wt_d = nc.dram_tensor("wt", [64, COUT], f16, kind="ExternalInput")
    out_d = nc.dram_tensor("out", [NPAIR, 128, PT], f16, kind="ExternalOutput")

    with TileContext(nc) as tc:
        with (
            tc.tile_pool(name="const", bufs=1) as cpool,
            tc.tile_pool(name="ft", bufs=4) as ftpool,
            tc.tile_pool(name="osb", bufs=4) as opool,
            tc.tile_pool(name="ps", bufs=4, space="PSUM") as pspool,
        ):
            # weights + coef first: small, one-time setup constants
            w_sb = cpool.tile([128, NCHUNK * COUT], f16, tag="w")
            wt_sb = cpool.tile([64, COUT], f16, tag="wt")
            coef_sb = cpool.tile([128, NCHUNK * PT], f16, tag="coef")

            nc.sync.dma_start(
                w_sb[:, :].rearrange("p (q o) -> p q o", q=NCHUNK), w_d[:]
            )
            nc.sync.dma_start(wt_sb[:, :], wt_d[:])
            nc.sync.dma_start(
                coef_sb[:, :].rearrange("p (q m) -> p q m", q=NCHUNK), coef_d[:]
            )
            # x streamed in 4-item blocks (per-block tiles so the bench
            # loop's reloads pipeline at block granularity)
            XB = 4
            xb = [cpool.tile([128, XB * PT], f16, tag=f"xb{b}", name=f"xb{b}")
                  for b in range(N // XB)]

            coef3d = coef_sb[:, :].rearrange("r (q m) -> r q m", q=NCHUNK)

            loop_cm = tc.For_i(0, loop_reps, 1) if loop_reps else contextlib.nullcontext()
            with loop_cm:
                for b in range(N // XB):
                    nc.sync.dma_start(
                        xb[b][:, :].rearrange("p (n m) -> p n m", n=XB),
                        x_d[:, b * XB:(b + 1) * XB, :],
                    )
                for pr in range(NPAIR):
                    items = (2 * pr, 2 * pr + 1)
                    fts = []
                    for j, it in enumerate(items):
                        ndve = dve_chunks[it % len(dve_chunks)]
                        ngp = NCHUNK - ndve
                        xs = xb[it // XB][:, (it % XB) * PT:(it % XB + 1) * PT]

                        ft = ftpool.tile([128, NCHUNK * PT], f16, name=f"ft{j}")
                        nc.vector.tensor_mul(
                            ft[:, : ndve * PT].rearrange(
                                "r (q p) -> r q p", q=ndve
                            ),
                            xs.unsqueeze(1).broadcast_to([128, ndve, PT]),
                            coef3d[:, :ndve, :],
                        )
                        if ngp:
                            nc.gpsimd.tensor_mul(
                                ft[:, ndve * PT:].rearrange(
                                    "r (q p) -> r q p", q=ngp
                                ),
                                xs.unsqueeze(1).broadcast_to([128, ngp, PT]),
                                coef3d[:, ndve:, :],
                            )
                        fts.append(ft)

                    ps = pspool.tile([128, PT], f32)
                    halves = (ps[0:64, :], ps[64:128, :])
                    for q in range(NCHUNK):
                        for j in range(2):
                            nc.tensor.matmul(
                                halves[j],
                                w_sb[:, q * COUT:(q + 1) * COUT],
                                fts[j][:, q * PT:(q + 1) * PT],
                                start=(q == 0),
                                stop=False,
                                tile_position=(0, 64 * j),
                                skip_group_check=True,
                            )
                    # k=0-real tail: K=32 matmul straight on x, row-tiled
                    # so the two halves' tails run concurrently
                    for j, it in enumerate(items):
                        nc.tensor.matmul(
                            halves[j],
                            wt_sb[32 * j:32 * (j + 1), :],
                            xb[it // XB][
                                32 * j:32 * (j + 1),
                                (it % XB) * PT:(it % XB + 1) * PT,
                            ],
                            start=False,
                            stop=True,
                            tile_position=(32 * j, 64 * j),
                            skip_group_check=True,
                        )
                    osb = opool.tile([128, PT], f16)
                    nc.scalar.copy(osb[:, :], ps[:, :])
                    nc.sync.dma_start(out_d[pr], osb[:, :])

    import bass_rust
    bass_rust.generate_event_semaphores(nc)
    return nc


def _make_nc_const(loop_reps=0):
    """Fast path: diag_L constant => coefs fold into W; one K=32 GEMM.

    x packed [128 = 4 n-sub x 32 c, N/4 n-grp, PT]: the 4 n-subs are 4
    row-groups of the PE array, pairs of items are 2 col-groups -> 8
    concurrent K=32 matmuls.
    """
    import contextlib

    import concourse.bass as bass
    import concourse.mybir as mybir
    from concourse.tile import TileContext

    f32 = mybir.dt.float32
    f16 = mybir.dt.float16
    NG = N // 4               # 8 n-groups of 4 items

    XBLK = (1, 3, 4)          # x-load taper: tiny first block -> early MMs
    OBLK = (3, 3, 2)          # out-batch taper (n-groups per DMA): short tail

    nc = bass.Bass()
    x_d = nc.dram_tensor("x", [128, NG, PT], f16, kind="ExternalInput")
    # block-diagonal [[Wfold, 0], [0, Wfold]] (K=64 packs 2 items/matmul),
    # replicated on both row halves
    w_d = nc.dram_tensor("w", [128, 128], f16, kind="ExternalInput")
    out_d = nc.dram_tensor("out", [128, NG * 2 * PT], f16, kind="ExternalOutput")

    with TileContext(nc) as tc:
        with (
            tc.tile_pool(name="const", bufs=1) as cpool,
            tc.tile_pool(name="osb", bufs=3) as opool,
            tc.tile_pool(name="ps", bufs=4, space="PSUM") as pspool,
        ):
            w_sb = cpool.tile([128, 128], f16, tag="w")
            xs = [cpool.tile([128, nb * PT], f16, tag=f"x{t}", name=f"x{t}")
                  for t, nb in enumerate(XBLK)]
            xoff = [sum(XBLK[:t]) for t in range(len(XBLK))]

            def xtile(b):     # -> (tile, col-offset) holding n-group b
                for t in range(len(XBLK) - 1, -1, -1):
                    if b >= xoff[t]:
                        return xs[t], (b - xoff[t]) * PT
                raise AssertionError

            nc.sync.dma_start(w_sb[:, :], w_d[:])

            loop_cm = tc.For_i(0, loop_reps, 1) if loop_reps else contextlib.nullcontext()
            with loop_cm:
                for t, nb in enumerate(XBLK):
                    nc.sync.dma_start(
                        xs[t][:, :].rearrange("p (u m) -> p u m", u=nb),
                        x_d[:, xoff[t]:xoff[t] + nb, :],
                    )
                bdone = 0
                for obatch, nob in enumerate(OBLK):   # nob n-groups per out-DMA
                    osb = opool.tile([128, nob * 2 * PT], f16)
                    for u in range(nob):
                        b = bdone + u                 # items 4b..4b+3
                        xt, xo = xtile(b)
                        pss = [pspool.tile([128, PT], f32, name=f"ps{k}")
                               for k in range(2)]
                        for k in range(2):            # K=64 pair: items 4b+2k, +2k+1
                            nc.tensor.matmul(
                                pss[k][:, :],
                                w_sb[64 * k:64 * (k + 1), :],
                                xt[64 * k:64 * (k + 1), xo:xo + PT],
                                start=True,
                                stop=True,
                                tile_position=(64 * k, 0),
                                skip_group_check=True,
                            )
                        # evac split: ScalarE + (idle) VectorE in parallel
                        o0 = osb[:, (2 * u) * PT:(2 * u + 1) * PT]
                        o1 = osb[:, (2 * u + 1) * PT:(2 * u + 2) * PT]
                        nc.scalar.copy(o0, pss[0][:, :])
                        nc.vector.tensor_copy(o1, pss[1][:, :])
                    nc.sync.dma_start(
                        out_d[:, bdone * 2 * PT:(bdone + nob) * 2 * PT],
                        osb[:, :],
                    )
                    bdone += nob

    import bass_rust
    bass_rust.generate_event_semaphores(nc)
    return nc


def _coefs(diag_L):
    """[16, M] nontrivial coefficient rows: Re(s^k), Im(s^k), k=1..8."""
    d = np.asarray(diag_L, dtype=np.float64)
    s = (d - 1j) ** 2 / (d * d + 1.0)
    out = np.empty((NGRP, d.shape[0]), dtype=np.float64)
    ck = s.copy()
    for k in range(ORDER):
        out[k] = ck.real
        out[ORDER + k] = ck.imag
        ck = ck * s
    return out


def _prep_host(x, real_weights, imag_weights, diag_L):
    x3 = np.asarray(x, dtype=np.float32).reshape(N, C, M)
    wr = np.asarray(real_weights, dtype=np.float64).reshape(KTOT, C, COUT)
    wi = np.asarray(imag_weights, dtype=np.float64).reshape(KTOT, C, COUT)
    coefs = _coefs(diag_L)

    # xr[g*32+c, n, p] = x[n, c, p]  (x4 over partition groups)
    xt = x3.transpose(1, 0, 2).astype(np.float16)           # [c, n, p]
    xr = np.broadcast_to(xt[None], (4, C, N, M)).reshape(128, N, M)

    # coef[gs*32+c, q, p] = coefs[4q+gs, p]
    cf = np.repeat(coefs.reshape(NCHUNK, 4, 1, M), C, axis=2)   # [q, gs, c, p]
    cf = np.ascontiguousarray(
        cf.transpose(1, 2, 0, 3).reshape(128, NCHUNK, M)
    ).astype(np.float16)

    # w[gs*32+c, q, o] = 2 * W_{4q+gs}[c, o]; groups = [re k=1..8, im k=1..8]
    wall = 2.0 * np.concatenate([wr[1:], wi[1:]], axis=0)       # [16, c, o]
    w = np.ascontiguousarray(
        wall.reshape(NCHUNK, 4, C, COUT).transpose(1, 2, 0, 3).reshape(
            128, NCHUNK, COUT
        )
    ).astype(np.float16)

    # tail: k=0-real (coef==1) on two row-group copies
    wt = np.ascontiguousarray(
        np.broadcast_to(2.0 * wr[0], (2, C, COUT)).reshape(64, COUT)
    ).astype(np.float16)
    return xr, cf, w, wt


def _prep_host_const(x, real_weights, imag_weights, diag_L):
    x3 = np.asarray(x, dtype=np.float32).reshape(N, C, M)
    wr = np.asarray(real_weights, dtype=np.float64).reshape(KTOT, C, COUT)
    wi = np.asarray(imag_weights, dtype=np.float64).reshape(KTOT, C, COUT)
    coefs = _coefs(diag_L[:1])                                  # [16, 1]
    # Wfold[c, o] = 2*Wr0 + sum_g coef_g * W_g
    wfold = 2.0 * wr[0] + np.tensordot(
        coefs[:, 0], 2.0 * np.concatenate([wr[1:], wi[1:]], axis=0), axes=(0, 0)
    )
    # x packed [sub*32+c, ngrp, p], n = ngrp*4 + sub
    xt = x3.reshape(N // 4, 4, C, M).transpose(1, 2, 0, 3)      # [sub, c, ng, p]
    xp = np.ascontiguousarray(xt.reshape(128, N // 4, M)).astype(np.float16)
    # block-diag [[Wfold, 0], [0, Wfold]] on each 64-row half
    wf = np.zeros((128, 128), dtype=np.float16)
    for half in (0, 1):
        wf[64 * half:64 * half + 32, :COUT] = wfold.astype(np.float16)
        wf[64 * half + 32:64 * half + 64, COUT:] = wfold.astype(np.float16)
    return xp, wf


def _in_map_general(args, i):
    xr, cf, w, wt = args
    sl = slice(PT * i, PT * (i + 1))
    return {
        "x": np.ascontiguousarray(xr[:, :, sl]),
        "coef": np.ascontiguousarray(cf[:, :, sl]),
        "w": w,
        "wt": wt,
    }


def _in_map_const(args, i):
    xp, wf = args
    sl = slice(PT * i, PT * (i + 1))
    return {"x": np.ascontiguousarray(xp[:, :, sl]), "w": wf}


def _assemble(results, const):
    out = np.empty((N, M, COUT), dtype=np.float32)
    for i in range(NCORES):
        o = np.asarray(results[i]["out"], dtype=np.float32)
        if const:
            # out[(half, o), (b, k, p)]: n = 4b + 2k + half
            o = o.reshape(2, COUT, 8, 2, PT).transpose(2, 3, 0, 4, 1)
        else:
            # out[pair, (half, o), p]: n = 2*pair + half
            o = o.reshape(NPAIR, 2, COUT, PT).transpose(0, 1, 3, 2)
        out[:, PT * i:PT * (i + 1), :] = o.reshape(N, PT, COUT)
    return out.reshape(N, MSIDE, MSIDE, COUT)


def kernel(x, real_weights, imag_weights, diag_L, vals, rows, cols):
    global LAST_RESULTS
    from concourse.bass_utils import run_bass_kernel_spmd

    diag = np.asarray(diag_L, dtype=np.float32)
    const_diag = bool(np.all(diag == diag[0]))
    if os.environ.get("CAYLEY_FORCE_GENERAL"):
        const_diag = False

    if const_diag:
        args = _prep_host_const(x, real_weights, imag_weights, diag)
        if "nc_const" not in _STATE:
            _STATE["nc_const"] = _make_nc_const()
        nc = _STATE["nc_const"]
        in_maps = [_in_map_const(args, i) for i in range(NCORES)]
    else:
        args = _prep_host(x, real_weights, imag_weights, diag)
        if "nc" not in _STATE:
            _STATE["nc"] = _make_nc()
        nc = _STATE["nc"]
        in_maps = [_in_map_general(args, i) for i in range(NCORES)]

    res = run_bass_kernel_spmd(nc, in_maps, list(range(NCORES)))
    LAST_RESULTS = res
    return _assemble(res.results, const_diag)
